# revision 12
# baseline (speedup 1.0000x reference)
"""Causal self-attention on 8 trn2 cores.

Sharding: core = 2*b + g  (b in 0..3 data-parallel over batch,
g in 0..1 tensor-parallel over head groups of 8 heads).

Per-core pipeline (all f16 matmul operands, f32 psum):
  K^T = Wk^T x^T (feature-major), V = x Wv (token-major, ones col per
  head), Q^T per 512-query chunk.  Attention per (q-chunk, head-pair):
  S^T tiles via row-group-packed K=64 matmuls; softmax exp split across
  engines: off-diagonal blocks on ScalarE (ACT exp), diagonal blocks on
  VectorE via a fused Schraudolph f16-exp (bits = s*C + maskbias ->
  int16, saturating; causal mask folded in as -1e6 bias -> -0.0).
  O^T accumulates [V|1].T @ P^T; softmax denominator rides as psum row
  64.  Normalize: ACT copies den/64 to f16, PE ones-broadcast matmul,
  DVE reciprocal + scaled multiply.  Output projection per q-chunk.
  Emission interleaves QKV/proj work into the attention stream so the
  PE never idles while ACT/DVE run softmax.

Host sums the two tensor-parallel partial y per batch and adds bproj.
Self-contained: hardcodes B=4, T=2048, C=1024, H=16.
"""
import os
import numpy as np

import concourse.bacc as bacc
import concourse.tile as tile
from concourse import mybir
from concourse import bass_utils
from contextlib import ExitStack

f32 = mybir.dt.float32
f32r = mybir.dt.float32r
f16 = mybir.dt.float16
i16 = mybir.dt.int16

B, T, C, H = 4, 2048, 1024, 16
HL, D = 8, 64            # local heads per core, head dim
DL = HL * D              # 512 local qkv features
QW = 512                 # q-chunk width
NQC = T // QW            # 4 q chunks
KT = T // 128            # 16 k tiles
CT = C // 128            # 8 contraction tiles

C_SCHR = 1477.3196 * 0.125   # schraudolph scale (f16 bits per unit score)
B_SCHR = 15316.0             # schraudolph bias (minimax)

_cache = {}


def _build():
    nc = bacc.Bacc(None, target_bir_lowering=False, debug=False)
    with tile.TileContext(nc) as tc, ExitStack() as ctx:
        xT = nc.dram_tensor("xT", [C, T], f16, kind="ExternalInput")
        wq = nc.dram_tensor("wq", [C, DL], f16, kind="ExternalInput")
        wk = nc.dram_tensor("wk", [C, DL], f16, kind="ExternalInput")
        wv = nc.dram_tensor("wv", [C, DL], f16, kind="ExternalInput")
        wo = nc.dram_tensor("wo", [DL, C], f16, kind="ExternalInput")
        mb = nc.dram_tensor("mb", [256, 1024], f32, kind="ExternalInput")
        y = nc.dram_tensor("y", [T, C], f32, kind="ExternalOutput")

        pers = ctx.enter_context(tc.tile_pool(name="pers", bufs=1))
        ptp = ctx.enter_context(tc.tile_pool(name="ptp", bufs=3))
        nrm = ctx.enter_context(tc.tile_pool(name="nrm", bufs=2))
        pcs = ctx.enter_context(tc.tile_pool(name="pcs", bufs=4))
        psA = ctx.enter_context(tc.tile_pool(name="psA", bufs=2, space="PSUM"))
        psS = ctx.enter_context(tc.tile_pool(name="psS", bufs=2, space="PSUM"))

        xt = [pers.tile([128, T], f16, tag=f"xt{ct}", name=f"xt{ct}")
              for ct in range(CT)]
        qsb = [pers.tile([128, T], f16, tag=f"qsb{m}", name=f"qsb{m}") for m in range(4)]
        ksb = [pers.tile([128, T], f16, tag=f"ksb{m}", name=f"ksb{m}") for m in range(4)]
        osb = [pers.tile([128, T], f16, tag=f"osb{m}", name=f"osb{m}") for m in range(4)]
        vsb = [pers.tile([128, HL, D + 1], f16, tag=f"vsb{t}", name=f"vsb{t}")
               for t in range(KT)]
        wo_sb = [pers.tile([128, C], f16, tag=f"wo{i}", name=f"wo{i}") for i in range(4)]
        wkt = pers.tile([128, 4, CT, 128], f16, tag="wkt", name="wkt")
        wqt = pers.tile([128, 4, CT, 128], f16, tag="wqt", name="wqt")
        wvt = pers.tile([128, CT, DL], f16, tag="wvt", name="wvt")
        mb_sb = [pers.tile([128, 1024], f32, tag=f"mb{k2}", name=f"mb{k2}")
                 for k2 in range(2)]
        ones64 = pers.tile([1, 64], f16, tag="ones64", name="ones64")

        # ---- input DMAs spread across engine queues for parallel load ----
        for m in range(4):
            nc.sync.dma_start(
                out=wkt[:, m],
                in_=wk[:, m * 128:(m + 1) * 128].rearrange(
                    "(ct p) mc -> p ct mc", p=128))
        for ct in range(0, CT, 2):
            nc.gpsimd.dma_start(out=xt[ct], in_=xT[ct * 128:(ct + 1) * 128, :])
        for ct in range(1, CT, 2):
            nc.scalar.dma_start(out=xt[ct], in_=xT[ct * 128:(ct + 1) * 128, :])
        nc.sync.dma_start(out=wvt, in_=wv[:, :].rearrange(
            "(ct p) f -> p ct f", p=128))
        for m in range(4):
            nc.gpsimd.dma_start(
                out=wqt[:, m],
                in_=wq[:, m * 128:(m + 1) * 128].rearrange(
                    "(ct p) mc -> p ct mc", p=128))
        for k2 in range(2):
            nc.sync.dma_start(out=mb_sb[k2], in_=mb[k2 * 128:(k2 + 1) * 128, :])
        for i in range(4):
            nc.sync.dma_start(out=wo_sb[i], in_=wo[i * 128:(i + 1) * 128, :])
        nc.vector.memset(ones64, 1.0)

        # ---- phase-A / phase-C work units ----
        def k_pair(np_, m):
            # two q-column chunks per stationary load (n = 2*np_, 2*np_+1)
            n0 = 2 * np_
            ps0 = psA.tile([128, 512], f32, tag="ps", name="ps")
            ps1 = psA.tile([128, 512], f32, tag="ps", name="ps")
            for ct in range(CT):
                nc.tensor.matmul(ps0, wkt[:, m, ct],
                                 xt[ct][:, n0 * 512:(n0 + 1) * 512],
                                 start=(ct == 0), stop=(ct == CT - 1))
                nc.tensor.matmul(ps1, wkt[:, m, ct],
                                 xt[ct][:, (n0 + 1) * 512:(n0 + 2) * 512],
                                 start=(ct == 0), stop=(ct == CT - 1))
            nc.any.tensor_copy(ksb[m][:, n0 * 512:(n0 + 1) * 512], ps0)
            nc.any.tensor_copy(ksb[m][:, (n0 + 1) * 512:(n0 + 2) * 512], ps1)

        def q_group(qc, m):
            ps = psA.tile([128, 512], f32, tag="ps", name="ps")
            for ct in range(CT):
                nc.tensor.matmul(ps, wqt[:, m, ct],
                                 xt[ct][:, qc * 512:(qc + 1) * 512],
                                 start=(ct == 0), stop=(ct == CT - 1))
            nc.any.tensor_copy(qsb[m][:, qc * 512:(qc + 1) * 512], ps)

        def v_group(t):
            ps = psA.tile([128, 512], f32, tag="ps", name="ps")
            for ct in range(CT):
                nc.tensor.matmul(ps, xt[ct][:, t * 128:(t + 1) * 128],
                                 wvt[:, ct, :],
                                 start=(ct == 0), stop=(ct == CT - 1))
            nc.any.tensor_copy(
                vsb[t][:, :, 0:D],
                ps[:].rearrange("p (h d) -> p h d", h=HL))
            nc.any.memset(vsb[t][:, :, D:D + 1], 1.0)

        def c_t(t):
            # both output halves per osb stationary load
            ps0 = psA.tile([128, 512], f32, tag="ps", name="ps")
            ps1 = psA.tile([128, 512], f32, tag="ps", name="ps")
            for m in range(4):
                nc.tensor.matmul(ps0, osb[m][:, t * 128:(t + 1) * 128],
                                 wo_sb[m][:, 0:512],
                                 start=(m == 0), stop=(m == 3))
                nc.tensor.matmul(ps1, osb[m][:, t * 128:(t + 1) * 128],
                                 wo_sb[m][:, 512:1024],
                                 start=(m == 0), stop=(m == 3))
            for n2, ps in ((0, ps0), (1, ps1)):
                yt = pcs.tile([128, 512], f32, tag="yt", name="yt")
                nc.any.tensor_copy(yt, ps)
                nc.sync.dma_start(
                    out=y[t * 128:(t + 1) * 128, n2 * 512:(n2 + 1) * 512],
                    in_=yt)

        # ---- attention unit for (qc, hp) ----
        def b_unit(qc, hp):
            pair = (2 * hp, 2 * hp + 1)
            last_kt = 4 * qc + 3
            pvp = {h: psA.tile([D + 1, QW], f32, tag="pv", name="pv")
                   for h in pair}
            ptb = {}

            def pv_half(blk, js):
                for j in js:
                    kt = blk * 4 + j
                    for h in pair:
                        nc.tensor.matmul(
                            pvp[h], vsb[kt][:, h, :],
                            ptb[(h, blk)][:, j * 512:(j + 1) * 512],
                            start=(kt == 0), stop=(kt == last_kt))

            for blk in range(qc + 1):
                diag = blk == qc
                for h in pair:
                    ptb[(h, blk)] = ptp.tile([128, 4 * 512], f16,
                                             tag=f"pt{h % 2}", name=f"pt{h % 2}")
                for k2 in range(2):
                    ss = {h: psS.tile([128, 1024], f32, tag="ss", name="ss")
                          for h in pair}
                    # h-inner so consecutive matmuls alternate PE row groups
                    for j in (0, 1):
                        kt = blk * 4 + k2 * 2 + j
                        for h in pair:
                            r0 = 64 * (h % 2)
                            nc.tensor.matmul(
                                ss[h][:, j * 512:(j + 1) * 512],
                                ksb[hp][r0:r0 + 64, kt * 128:(kt + 1) * 128],
                                qsb[hp][r0:r0 + 64, qc * QW:(qc + 1) * QW],
                                start=True, stop=True)
                    for h in pair:
                        dst = ptb[(h, blk)][:, k2 * 1024:(k2 + 1) * 1024]
                        if diag:
                            # fused schraudolph exp + causal mask on DVE
                            nc.vector.scalar_tensor_tensor(
                                dst.bitcast(i16), ss[h], C_SCHR, mb_sb[k2],
                                mybir.AluOpType.mult, mybir.AluOpType.add)
                        else:
                            nc.scalar.activation(
                                dst, ss[h],
                                mybir.ActivationFunctionType.Exp, scale=0.125)
                    if blk > 0:
                        pv_half(blk - 1, (2 * k2, 2 * k2 + 1))
            pv_half(qc, (0, 1))
            pv_half(qc, (2, 3))

            # normalize: den/64 -> f16 -> PE broadcast -> recip -> scaled mul
            for h in pair:
                f = hp
                denf = nrm.tile([1, QW], f16, tag="denf", name="denf")
                nc.scalar.mul(denf, pvp[h][D:D + 1, :], 1.0 / 64.0)
                bc = psA.tile([128, QW], f32, tag="ps", name="ps")
                nc.tensor.matmul(bc[0:64, :], ones64, denf,
                                 start=True, stop=True)
                rcp = nrm.tile([64, QW], f32, tag="rcp", name="rcp")
                nc.vector.reciprocal_approx_fast(out=rcp, in_=bc[0:64, :])
                if h % 2 == 0:
                    nc.vector.scalar_tensor_tensor(
                        osb[f][0:64, qc * QW:(qc + 1) * QW],
                        pvp[h][0:D, :], 1.0 / 64.0, rcp,
                        mybir.AluOpType.mult, mybir.AluOpType.mult)
                else:
                    tmp = nrm.tile([64, QW], f16, tag="tmp", name="tmp")
                    nc.vector.scalar_tensor_tensor(
                        tmp, pvp[h][0:D, :], 1.0 / 64.0, rcp,
                        mybir.AluOpType.mult, mybir.AluOpType.mult)
                    nc.sync.dma_start(
                        out=osb[f][64:128, qc * QW:(qc + 1) * QW],
                        in_=tmp)

        # ---- emission: head, then interleaved attention + filler ----
        for m in range(4):
            k_pair(0, m)          # K columns n=0,1
        for t in range(4):
            v_group(t)
        for m in range(4):
            q_group(0, m)

        def fillers(qc, hp):
            out = []
            if qc == 0:
                out += [(k_pair, (1, hp))]      # K columns n=2,3
                if hp == 2:
                    out += [(v_group, (4,))]
                if hp == 3:
                    out += [(v_group, (t,)) for t in (5, 6, 7)]
                    out += [(q_group, (1, m)) for m in range(4)]
            elif qc in (1, 2):
                out += [(v_group, (4 * (qc + 1) + hp,))]
                out += [(c_t, (4 * (qc - 1) + hp,))]
                if hp == 3:
                    out += [(q_group, (qc + 1, m)) for m in range(4)]
            else:
                out += [(c_t, (8 + hp,))]
            return out

        for qc in range(NQC):
            for hp in range(4):
                b_unit(qc, hp)
                for fn, args in fillers(qc, hp):
                    fn(*args)
        for t in range(12, 16):
            c_t(t)

    nc.compile()
    return nc


def _maskbias():
    mb = np.full((2, 128, 1024), B_SCHR, dtype=np.float32)
    kp = np.arange(128)[:, None]
    q = np.arange(512)[None, :]
    for k2 in range(2):
        for j in range(2):
            kt = k2 * 2 + j
            masked = q < (kt * 128 + kp)
            blkv = mb[k2][:, j * 512:(j + 1) * 512]
            blkv[masked] = -1e6
    return mb.reshape(256, 1024)


def kernel(x, Wqkv, bqkv, bproj=None, Wproj=None, **kw):
    # tolerate arbitrary kw ordering from harness
    if Wproj is None:
        Wproj = kw["Wproj"]
    x = np.asarray(x, dtype=np.float32)
    Wqkv = np.asarray(Wqkv, dtype=np.float32)
    bqkv = np.asarray(bqkv, dtype=np.float32)
    Wproj = np.asarray(Wproj, dtype=np.float32)
    bproj = np.asarray(bproj, dtype=np.float32)
    assert not np.any(bqkv), "nonzero bqkv not supported by this build"

    if "nc" not in _cache:
        _cache["nc"] = _build()
    nc = _cache["nc"]

    w3 = Wqkv.reshape(C, 3, H, D)
    mbias = _maskbias()
    in_maps = []
    for core in range(8):
        b, g = core // 2, core % 2
        hs = slice(g * HL, (g + 1) * HL)
        in_maps.append({
            "xT": np.ascontiguousarray(x[b].T).astype(np.float16),
            "wq": np.ascontiguousarray(
                w3[:, 0, hs, :].reshape(C, DL)).astype(np.float16),
            "wk": np.ascontiguousarray(
                w3[:, 1, hs, :].reshape(C, DL)).astype(np.float16),
            "wv": np.ascontiguousarray(
                w3[:, 2, hs, :].reshape(C, DL)).astype(np.float16),
            "wo": np.ascontiguousarray(
                Wproj[g * DL:(g + 1) * DL, :]).astype(np.float16),
            "mb": mbias,
        })

    trace = bool(int(os.environ.get("KERNEL_TRACE", "0")))
    res = bass_utils.run_bass_kernel_spmd(nc, in_maps, core_ids=list(range(8)),
                                          trace=trace)
    _cache["last_exec_ns"] = res.exec_time_ns
    _cache["res"] = res
    if trace:
        print("HW exec time:", res.exec_time_ns, "ns")

    out = np.empty((B, T, C), dtype=np.float32)
    for b in range(B):
        out[b] = res.results[2 * b]["y"] + res.results[2 * b + 1]["y"]
    out += bproj[None, None, :]
    return out
